# revision 28
# baseline (speedup 1.0000x reference)
"""DGCNN-simple Trainium2 kernel (v2).

Strategy (8 NeuronCores, B=4 samples):
  core c -> sample b = c//2, query half h = c%2 (2048 queries each).
  Per EdgeConv, the conv+BN+LReLU+max over K neighbors is folded using
  LReLU monotonicity:
     x_out(o,n) = LReLU( max_k P(o, j_k(n)) + Q(o,n) )
  with P = (s*A) @ x  (neighbor part) and Q = (s*(B-A)) @ x + t (center
  part), where A/B are the W column blocks and s,t the BN affine.

  KNN v2: pd = 2 x_q.x_c - |x_q|^2 - 1 - |x_c|^2 computed on the PE in
  float32r (norms folded in as extra contraction rows; the -1 keeps all
  values strictly <= -1 so every packed value is a normal float).  The
  column index is packed into the low 12 mantissa bits (pd |= iota on
  gpsimd; local 10-bit iota + quarter base OR'd via the scalar port), so
  max8 values carry their indices and MaxIndex is never needed.  Top-20 =
  top-8 of each of 8 512-wide chunks (8 max8 ops, same total scan width
  as ONE full pass) -> 64 candidates -> exact top-24 of the candidates
  (3 max8 + 2 match_replace on 64 elems).  The candidate union misses a
  true top-20 member only when >8 of them fall in one 512-chunk
  (p ~ 2e-3 per row; harmless substitution of the ~21st neighbor).
  Neighbor gather: ONE batched indirect DMA (2560 rows of P^T) per tile;
  max over k via a single strided tensor_reduce.
  x1 halves are exchanged between core pairs with an AllGather.
"""

import numpy as np
import concourse.bass as bass
import concourse.bacc as bacc
import concourse.mybir as mybir
import concourse.tile as tile
from concourse.bass_utils import run_bass_kernel_spmd
from concourse.masks import make_identity

N = 4096
K = 20
B = 4
EPS = 1e-5
SLOPE = 0.2
NCORES = 8
HALF = N // 2
P128 = 128
NT = HALF // P128  # query tiles per core
F32 = mybir.dt.float32
F32R = mybir.dt.float32r
I32 = mybir.dt.int32
NCHUNK = 8  # KNN candidate chunks per 4096 row
CW = N // NCHUNK  # chunk width

_BUILT = {}


def _lrelu(nc, out_ap, in_ap):
    # lrelu(x) = max(0.2*x, x) in one DVE op
    nc.vector.scalar_tensor_tensor(out=out_ap, in0=in_ap, scalar=SLOPE, in1=in_ap,
                                   op0=mybir.AluOpType.mult, op1=mybir.AluOpType.max)


def _knn_topk(nc, sb, pd_sb, iota, mask24, maskhi):
    """pd_sb (128, N) fp32, all values <= -1. Packs the global column index
    into the low 12 mantissa bits: (pd & ~0xFFF) | iota, one DVE STT pass.
    Then per-chunk top-8, then exact top-24 of the 64 candidates.
    Returns (128, 24) int32 index tile."""
    pv = pd_sb[:].bitcast(I32)
    nc.vector.scalar_tensor_tensor(out=pv, in0=pv, scalar=maskhi[:, 0:1],
                                   in1=iota[:],
                                   op0=mybir.AluOpType.bitwise_and,
                                   op1=mybir.AluOpType.bitwise_or)
    cand = sb.tile([P128, NCHUNK * 8], F32, tag="cand")
    for c in range(NCHUNK):
        nc.vector.max(out=cand[:, 8 * c:8 * c + 8], in_=pd_sb[:, CW * c:CW * (c + 1)])
    p24 = sb.tile([P128, 24], F32, tag="p24")
    for r in range(3):
        nc.vector.max(out=p24[:, 8 * r:8 * r + 8], in_=cand[:])
        if r < 2:
            nc.vector.match_replace(out=cand[:], in_to_replace=p24[:, 8 * r:8 * r + 8],
                                    in_values=cand[:], imm_value=-1e30)
    idx = sb.tile([P128, 24], I32, tag="idx")
    nc.vector.tensor_tensor(out=idx[:], in0=p24[:].bitcast(I32), in1=mask24[:],
                            op=mybir.AluOpType.bitwise_and)
    return idx


def _edgeconv_phase(nc, tc, pools, *, lhs, rhs, cdim, ptab_dram, bq_rhs,
                    bq_cdim, lhsq, out_cb, identity, iota, mask24, maskhi):
    """One EdgeConv: for each of NT query tiles compute pd (f32r matmul),
    packed top-20, one batched gather of P rows, max over k, add Q, LReLU.
    out_cb(t, xt) is called with the (128, 65) query-major result tile."""
    sb, ps, pd_ps, stg = pools
    for t in range(NT):
        q0 = t * P128
        # pd in 4 psum quarters of 1024 cols -> sbuf
        pd_sb = sb.tile([P128, N], F32, tag="pd_sb")
        for quar in range(4):
            pq = pd_ps.tile([P128, 1024], F32, tag="pdq")
            for j in range(2):
                c0 = quar * 1024 + j * 512
                nc.tensor.matmul(pq[:, j * 512:(j + 1) * 512],
                                 lhs[0:cdim, q0:q0 + P128],
                                 rhs[0:cdim, c0:c0 + 512],
                                 start=True, stop=True)
            nc.scalar.copy(pd_sb[:, quar * 1024:(quar + 1) * 1024], pq[:])
        idx = _knn_topk(nc, sb, pd_sb, iota, mask24, maskhi)
        # gather K neighbor rows of P^T (N, 64) per query; the HW indirect
        # DMA consumes ONE index per partition, so one DMA per k
        gath = sb.tile([P128, K * 64], F32, tag="gath")
        for k in range(K):
            nc.gpsimd.indirect_dma_start(
                out=gath[:, k * 64:(k + 1) * 64],
                out_offset=None,
                in_=ptab_dram[:],
                in_offset=bass.IndirectOffsetOnAxis(ap=idx[:, k:k + 1], axis=0),
            )
        # max over k: view gath (p, k, o) as (p, o, k), reduce innermost k
        m = sb.tile([P128, 64], F32, tag="gmx", name="gmx")
        nc.vector.tensor_reduce(
            out=m[:],
            in_=gath[:].rearrange("p (k o) -> p o k", o=64),
            axis=mybir.AxisListType.X, op=mybir.AluOpType.max)
        # Q part into psum, then accumulate the gather-max via identity matmul
        qp = ps.tile([P128, 64], F32, tag="ps")
        nc.tensor.matmul(qp[:], lhsq[0:bq_cdim, q0:q0 + P128], bq_rhs[0:bq_cdim, :],
                         start=True, stop=False)
        nc.tensor.matmul(qp[:], identity[:], m[:], start=False, stop=True)
        xt = sb.tile([P128, 65], F32, tag="xoq", name="xoq")
        nc.scalar.copy(xt[:, 0:64], qp[:])
        _lrelu(nc, xt[:, 0:64], xt[:, 0:64])
        out_cb(t, xt)


def _ptable(nc, pools, a_rhs, cdim, src, identity, ptab_dram, ptab_stage):
    """P = A^T-weights @ src -> transpose -> DRAM table (N, 64).
    a_rhs: (cdim, 64) stationary; src: (cdim, N)."""
    sb, ps, pd_ps, stg = pools
    for j in range(N // 512):
        pp = ps.tile([64, 512], F32, tag="ps")
        nc.tensor.matmul(pp[:], a_rhs[0:cdim, :], src[0:cdim, j * 512:(j + 1) * 512],
                         start=True, stop=True)
        p512 = stg.tile([64, 512], F32, tag="p512", name="p512")
        nc.scalar.copy(p512[:], pp[:])
        for i in range(4):
            j1 = 4 * j + i
            tp = ps.tile([P128, P128], F32, tag="ps")
            nc.tensor.transpose(tp[:, 0:64], p512[:, i * P128:(i + 1) * P128],
                                identity[0:64, 0:64])
            nc.scalar.copy(ptab_stage[:, j1 * 64:(j1 + 1) * 64], tp[:, 0:64])
    nc.sync.dma_start(
        ptab_dram[:].rearrange("(j1 j0) o -> j0 j1 o", j0=P128),
        ptab_stage[:].rearrange("p (j1 o) -> p j1 o", o=64))


def build():
    nc = bacc.Bacc(None, target_bir_lowering=False)
    dt = F32
    # ---- per-core inputs ----
    lhsA_d = nc.dram_tensor("lhsA_d", [7, HALF], dt, kind="ExternalInput")
    rhsA_d = nc.dram_tensor("rhsA_d", [7, N], dt, kind="ExternalInput")
    ones_d = nc.dram_tensor("ones_d", [2, N], dt, kind="ExternalInput")
    a1t = nc.dram_tensor("a1t", [2, 64], dt, kind="ExternalInput")
    b1t = nc.dram_tensor("b1t", [7, 64], dt, kind="ExternalInput")
    a2t = nc.dram_tensor("a2t", [64, 64], dt, kind="ExternalInput")
    b2t = nc.dram_tensor("b2t", [66, 64], dt, kind="ExternalInput")
    w5t = nc.dram_tensor("w5t", [128, 128], dt, kind="ExternalInput")
    t5 = nc.dram_tensor("t5", [128, 1], dt, kind="ExternalInput")
    w6at = nc.dram_tensor("w6at", [128, 256], dt, kind="ExternalInput")
    w6bt = nc.dram_tensor("w6bt", [128, 256], dt, kind="ExternalInput")
    t6 = nc.dram_tensor("t6", [128, 2], dt, kind="ExternalInput")
    w9t = nc.dram_tensor("w9t", [128, 2], dt, kind="ExternalInput")
    out = nc.dram_tensor("out", [1, HALF], dt, kind="ExternalOutput")

    with tile.TileContext(nc) as tc:
        with tc.tile_pool(name="sb", bufs=2) as sb, \
             tc.tile_pool(name="stg", bufs=1) as stg, \
             tc.tile_pool(name="sbp", bufs=1) as sbp, \
             tc.tile_pool(name="ps", bufs=2, space="PSUM") as ps, \
             tc.tile_pool(name="pdps", bufs=3, space="PSUM") as pd_ps, \
             tc.tile_pool(name="dram", bufs=1, space="DRAM") as dram:
            pools = (sb, ps, pd_ps, stg)
            ident = sbp.tile([P128, P128], F32, tag="ident")
            make_identity(nc, ident[:])
            iota = sbp.tile([P128, N], I32, tag="iota")
            nc.gpsimd.iota(iota[:], pattern=[[1, N]], base=0,
                           channel_multiplier=0)
            mask24 = sbp.tile([P128, 24], I32, tag="mask24")
            nc.vector.memset(mask24[:], 4095)
            maskhi = sbp.tile([P128, 1], I32, tag="maskhi")
            nc.gpsimd.iota(maskhi[:], pattern=[[1, 1]], base=-4096,
                           channel_multiplier=0)

            # ---------- phase A prep ----------
            # pd(q,m) = sum_c lhs_r[c,q]*rhs_r[c,m], cdim=7, rows built on
            # the HOST: lhsA_d = [2x_q; -x_q^2; 1;1;1],
            # rhsA_d = [x_m; 1; 1; -x_m^2; -1]
            # (the -1 keeps every pd <= -1: normal floats, packable)
            lhsA = sbp.tile([66, HALF], F32, tag="lhsAB")
            rhsA = sbp.tile([66, N], F32, tag="rhsAB")
            nc.sync.dma_start(lhsA[0:7, :], lhsA_d[:])
            nc.sync.dma_start(rhsA[0:7, :], rhsA_d[:])


            # ---------- load small weights (after the x-critical chain) ----
            def load(pool, dr, shape, tag):
                t = pool.tile(list(shape), F32, tag=tag)
                nc.sync.dma_start(t[:], dr[:])
                return t

            a1s = load(sbp, a1t, (2, 64), "a1")
            b1s = load(sbp, b1t, (7, 64), "b1")
            a2s = load(sbp, a2t, (64, 64), "a2")
            b2s = load(sbp, b2t, (66, 64), "b2")
            w5s = load(sbp, w5t, (128, 128), "w5")
            t5s = load(sbp, t5, (128, 1), "t5")
            w6as = load(sbp, w6at, (128, 256), "w6a")
            w6bs = load(sbp, w6bt, (128, 256), "w6b")
            t6s = load(sbp, t6, (128, 2), "t6")
            w9s = load(sbp, w9t, (128, 2), "w9")

            # ---------- P1 table ----------
            p1d = dram.tile([N, 64], F32, tag="p1d")
            pstage = stg.tile([P128, (N // P128) * 64], F32, tag="pstage")
            _ptable(nc, pools, a1s, 2, rhsA, ident, p1d, pstage)

            # ---------- EdgeConv 1 ----------
            # x1 q-major -> (sq-norm col, with extra -1) -> transpose into
            # x1aug (66, HALF)
            x1aug = sbp.tile([66, HALF], F32, tag="x1aug")

            def cb1(t, xt):
                sq = sb.tile([P128, 64], F32, tag="sq64", name="sq64")
                nc.vector.tensor_mul(sq[:], xt[:, 0:64], xt[:, 0:64])
                nc.vector.tensor_reduce(out=xt[:, 64:65], in_=sq[:],
                                        axis=mybir.AxisListType.X,
                                        op=mybir.AluOpType.add, negate=True)
                nc.vector.tensor_scalar_add(xt[:, 64:65], xt[:, 64:65], -1.0)
                tp = ps.tile([P128, P128], F32, tag="ps", name="tp1")
                nc.tensor.transpose(tp[0:65, :], xt[:, 0:65], ident[:])
                nc.scalar.copy(x1aug[0:65, t * P128:(t + 1) * P128], tp[0:65, :])

            _edgeconv_phase(nc, tc, pools, lhs=lhsA, rhs=rhsA, cdim=7,
                            ptab_dram=p1d, bq_rhs=b1s, bq_cdim=7, lhsq=lhsA,
                            out_cb=cb1, identity=ident, iota=iota, mask24=mask24, maskhi=maskhi)
            nc.sync.dma_start(x1aug[65:66, :], ones_d[0:1, 0:HALF])

            # ---------- exchange halves (AllGather over core pairs) ----------
            ccin = dram.tile([66, HALF], F32, tag="ccin")
            ccout = dram.tile([132, HALF], F32, tag="ccout")
            nc.sync.dma_start(ccin[:], x1aug[0:66, :])
            nc.gpsimd.collective_compute(
                "AllGather", mybir.AluOpType.bypass,
                replica_groups=[[0, 1], [2, 3], [4, 5], [6, 7]],
                ins=[ccin.opt()], outs=[ccout.opt()])

            # ---------- phase B prep ----------
            # collective-independent work first so engines stay busy during
            # the exchange: lhsB = [2*x1; norm; ones] via Act scale-2 copy
            lhsB = sbp.tile([66, HALF], F32, tag="lhsAB")
            nc.scalar.activation(lhsB[0:64, :], x1aug[0:64, :],
                                 mybir.ActivationFunctionType.Copy, scale=2.0)
            nc.scalar.copy(lhsB[64:66, :], x1aug[64:66, :])
            xcown = sbp.tile([P128, HALF], F32, tag="xcown")
            nc.scalar.copy(xcown[0:64, :], x1aug[0:64, :])

            # rhsB (66, N): [x1_full; ones; -xx2_full - 1]  (reuses rhsA slot)
            # DMAs spread over engine queues so they issue in parallel
            rhsB = sbp.tile([66, N], F32, tag="rhsAB")
            nc.sync.dma_start(rhsB[0:64, 0:HALF], ccout[0:64, :])
            nc.scalar.dma_start(rhsB[0:64, HALF:N], ccout[66:130, :])
            nc.scalar.dma_start(rhsB[65:66, 0:HALF], ccout[64:65, :])
            nc.sync.dma_start(rhsB[65:66, HALF:N], ccout[130:131, :])
            nc.sync.dma_start(rhsB[64:65, :], ones_d[0:1, :])

            # ---------- P2 table ----------
            p2d = dram.tile([N, 64], F32, tag="p2d")
            _ptable(nc, pools, a2s, 64, rhsB, ident, p2d, pstage)

            # conv5 + global max pool is fused per 512-chunk into cb2:
            # chunk j runs as soon as query tiles 4j..4j+3 have landed
            gpart = stg.tile([P128, 4], F32, tag="gpart")

            def conv5_chunk(j):
                pp = ps.tile([P128, 512], F32, tag="ps")
                nc.tensor.matmul(pp[:], w5s[:], xcown[:, j * 512:(j + 1) * 512],
                                 start=True, stop=True)
                h5c = sb.tile([P128, 512], F32, tag="h5c", name="h5c")
                nc.scalar.activation(h5c[:], pp[:],
                                     mybir.ActivationFunctionType.Identity,
                                     bias=t5s[:, 0:1])
                _lrelu(nc, h5c[:], h5c[:])
                nc.vector.tensor_reduce(out=gpart[:, j:j + 1], in_=h5c[:],
                                        axis=mybir.AxisListType.X,
                                        op=mybir.AluOpType.max)

            def cb2(t, xt):
                tp = ps.tile([P128, P128], F32, tag="ps", name="tp2")
                nc.tensor.transpose(tp[0:64, :], xt[:, 0:64], ident[:])
                x2s = sb.tile([64, P128], F32, tag="x2s", name="x2s")
                nc.scalar.copy(x2s[:], tp[0:64, :])
                nc.sync.dma_start(xcown[64:128, t * P128:(t + 1) * P128], x2s[:])
                if t % 4 == 3:
                    conv5_chunk(t // 4)

            _edgeconv_phase(nc, tc, pools, lhs=lhsB, rhs=rhsB, cdim=66,
                            ptab_dram=p2d, bq_rhs=b2s, bq_cdim=66, lhsq=x1aug,
                            out_cb=cb2, identity=ident, iota=iota, mask24=mask24, maskhi=maskhi)

            # ---------- tail ----------
            gown = sb.tile([P128, 1], F32, tag="gown")
            nc.vector.tensor_reduce(out=gown[:], in_=gpart[:, 0:4],
                                    axis=mybir.AxisListType.X, op=mybir.AluOpType.max)
            gin = dram.tile([P128, 1], F32, tag="gin")
            gout = dram.tile([256, 1], F32, tag="gout")
            nc.sync.dma_start(gin[:], gown[:])
            nc.gpsimd.collective_compute(
                "AllGather", mybir.AluOpType.bypass,
                replica_groups=[[0, 1], [2, 3], [4, 5], [6, 7]],
                ins=[gin.opt()], outs=[gout.opt()])
            # W6b matmuls do not need gmax: issue them on the PE while the
            # collective runs, into wide psum tiles from the pd pool
            p6 = []
            for c in range(2):
                for jh in range(2):
                    pq6 = pd_ps.tile([P128, 1024], F32, tag="pdq",
                                     name=f"p6_{c}{jh}")
                    for j2 in range(2):
                        j = jh * 2 + j2
                        nc.tensor.matmul(pq6[:, j2 * 512:(j2 + 1) * 512],
                                         w6bs[:, c * 128:(c + 1) * 128],
                                         xcown[:, j * 512:(j + 1) * 512],
                                         start=True, stop=True)
                    p6.append(pq6)
            gboth = sb.tile([P128, 2], F32, tag="gboth")
            nc.sync.dma_start(gboth[:, 0:1], gout[0:P128, :])
            nc.sync.dma_start(gboth[:, 1:2], gout[P128:256, :])
            gmax = sb.tile([P128, 1], F32, tag="gmax")
            nc.vector.tensor_reduce(out=gmax[:], in_=gboth[:],
                                    axis=mybir.AxisListType.X, op=mybir.AluOpType.max)
            bias6 = sb.tile([P128, 2], F32, tag="bias6")
            for c in range(2):
                vp = ps.tile([P128, 64], F32, tag="ps")
                nc.tensor.matmul(vp[:, 0:1], w6as[:, c * 128:(c + 1) * 128], gmax[:],
                                 start=True, stop=True)
                nc.vector.tensor_add(out=bias6[:, c:c + 1], in0=vp[:, 0:1],
                                     in1=t6s[:, c:c + 1])
            h6t = []
            for c in range(2):
                for jh in range(2):
                    sl6 = sb.tile([P128, 1024], F32, tag=f"h6c{c}", name=f"h6c{c}")
                    nc.scalar.activation(sl6[:], p6[c * 2 + jh][:],
                                         mybir.ActivationFunctionType.Identity,
                                         bias=bias6[:, c:c + 1])
                    _lrelu(nc, sl6[:], sl6[:])
                    h6t.append(sl6)
            osb = sb.tile([1, HALF], F32, tag="osb")
            for jh in range(2):
                for j2 in range(2):
                    pp9 = ps.tile([1, 512], F32, tag="ps")
                    for c in range(2):
                        nc.tensor.matmul(pp9[:], w9s[:, c:c + 1],
                                         h6t[c * 2 + jh][:, j2 * 512:(j2 + 1) * 512],
                                         start=(c == 0), stop=(c == 1))
                    nc.scalar.copy(osb[:, (jh * 2 + j2) * 512:(jh * 2 + j2 + 1) * 512],
                                   pp9[:])
            nc.sync.dma_start(out[:], osb[:])

    nc.finalize()
    return nc


def _fold_weights(i):
    f = np.float32
    o = {}
    s1 = (i["g1"] / np.sqrt(i["v1"] + EPS)).astype(f)
    A1 = (i["W1"][:, 0:2] * s1[:, None]).astype(f)
    B1 = ((i["W1"][:, 2:4] - i["W1"][:, 0:2]) * s1[:, None]).astype(f)
    t1 = (i["b1"] - i["m1"] * s1).astype(f)
    o["a1t"] = np.ascontiguousarray(A1.T)
    b1t = np.zeros((7, 64), f)
    b1t[0:2] = 0.5 * B1.T
    b1t[4] = t1
    o["b1t"] = b1t
    s2 = (i["g2"] / np.sqrt(i["v2"] + EPS)).astype(f)
    A2 = (i["W2"][:, 0:64] * s2[:, None]).astype(f)
    B2 = ((i["W2"][:, 64:128] - i["W2"][:, 0:64]) * s2[:, None]).astype(f)
    t2 = (i["b2"] - i["m2"] * s2).astype(f)
    o["a2t"] = np.ascontiguousarray(A2.T)
    b2t = np.zeros((66, 64), f)
    b2t[0:64] = B2.T
    b2t[65] = t2
    o["b2t"] = b2t
    s5 = (i["g5"] / np.sqrt(i["v5"] + EPS)).astype(f)
    W5s = (i["W5"] * s5[:, None]).astype(f)
    o["w5t"] = np.ascontiguousarray(W5s.T)
    o["t5"] = (i["b5"] - i["m5"] * s5).astype(f).reshape(128, 1)
    s6 = (i["g6"] / np.sqrt(i["v6"] + EPS)).astype(f)
    W6s = (i["W6"] * s6[:, None]).astype(f)
    o["w6at"] = np.ascontiguousarray(W6s[:, 0:128].T)
    o["w6bt"] = np.ascontiguousarray(W6s[:, 128:256].T)
    t6 = (i["b6"] - i["m6"] * s6).astype(f)
    o["t6"] = np.ascontiguousarray(t6.reshape(2, 128).T)
    o["w9t"] = np.ascontiguousarray(i["W9"].reshape(2, 128).T)
    ones_d = np.ones((2, N), f)
    ones_d[1] = -1.0
    o["ones_d"] = ones_d
    return o


def _core_inputs(w, x, c):
    b, h = c // 2, c % 2
    m = dict(w)
    xb = x[b]
    xqh = xb[:, h * HALF:(h + 1) * HALF]
    lhsA_d = np.empty((7, HALF), np.float32)
    lhsA_d[0:2] = 2.0 * xqh
    lhsA_d[2:4] = -(xqh * xqh)
    lhsA_d[4:7] = 1.0
    rhsA_d = np.empty((7, N), np.float32)
    rhsA_d[0:2] = xb
    rhsA_d[2:4] = 1.0
    rhsA_d[4:6] = -(xb * xb)
    rhsA_d[6] = -1.0
    m["lhsA_d"] = lhsA_d
    m["rhsA_d"] = rhsA_d
    return m


def kernel(**inputs):
    inputs = {k: np.asarray(v, np.float32) for k, v in inputs.items()}
    if "nc" not in _BUILT:
        _BUILT["nc"] = build()
    nc = _BUILT["nc"]
    w = _fold_weights(inputs)
    x = inputs["x"]
    in_maps = [_core_inputs(w, x, c) for c in range(NCORES)]
    res = run_bass_kernel_spmd(nc, in_maps, core_ids=list(range(NCORES)))
    out = np.zeros((B, N), np.float32)
    for c in range(NCORES):
        b, h = c // 2, c % 2
        out[b, h * HALF:(h + 1) * HALF] = res.results[c]["out"][0]
    return out


# revision 29
# speedup vs baseline: 1.0166x; 1.0166x over previous
"""DGCNN-simple Trainium2 kernel (v2).

Strategy (8 NeuronCores, B=4 samples):
  core c -> sample b = c//2, query half h = c%2 (2048 queries each).
  Per EdgeConv, the conv+BN+LReLU+max over K neighbors is folded using
  LReLU monotonicity:
     x_out(o,n) = LReLU( max_k P(o, j_k(n)) + Q(o,n) )
  with P = (s*A) @ x  (neighbor part) and Q = (s*(B-A)) @ x + t (center
  part), where A/B are the W column blocks and s,t the BN affine.

  KNN v2: pd = 2 x_q.x_c - |x_q|^2 - 1 - |x_c|^2 computed on the PE in
  float32r (norms folded in as extra contraction rows; the -1 keeps all
  values strictly <= -1 so every packed value is a normal float).  The
  column index is packed into the low 12 mantissa bits (pd |= iota on
  gpsimd; local 10-bit iota + quarter base OR'd via the scalar port), so
  max8 values carry their indices and MaxIndex is never needed.  Top-20 =
  top-8 of each of 8 512-wide chunks (8 max8 ops, same total scan width
  as ONE full pass) -> 64 candidates -> exact top-24 of the candidates
  (3 max8 + 2 match_replace on 64 elems).  The candidate union misses a
  true top-20 member only when >8 of them fall in one 512-chunk
  (p ~ 2e-3 per row; harmless substitution of the ~21st neighbor).
  Neighbor gather: ONE batched indirect DMA (2560 rows of P^T) per tile;
  max over k via a single strided tensor_reduce.
  x1 halves are exchanged between core pairs with an AllGather.
"""

import numpy as np
import concourse.bass as bass
import concourse.bacc as bacc
import concourse.mybir as mybir
import concourse.tile as tile
from concourse.bass_utils import run_bass_kernel_spmd
from concourse.masks import make_identity

N = 4096
K = 20
B = 4
EPS = 1e-5
SLOPE = 0.2
NCORES = 8
HALF = N // 2
P128 = 128
NT = HALF // P128  # query tiles per core
F32 = mybir.dt.float32
F32R = mybir.dt.float32r
I32 = mybir.dt.int32
NCHUNK = 8  # KNN candidate chunks per 4096 row
CW = N // NCHUNK  # chunk width

_BUILT = {}


def _lrelu(nc, out_ap, in_ap):
    # lrelu(x) = max(0.2*x, x) in one DVE op
    nc.vector.scalar_tensor_tensor(out=out_ap, in0=in_ap, scalar=SLOPE, in1=in_ap,
                                   op0=mybir.AluOpType.mult, op1=mybir.AluOpType.max)


def _knn_topk(nc, sb, pd_sb, iota, mask24, maskhi):
    """pd_sb (128, N) fp32, all values <= -1. Packs the global column index
    into the low 12 mantissa bits: (pd & ~0xFFF) | iota, one DVE STT pass.
    Then per-chunk top-8, then exact top-24 of the 64 candidates.
    Returns (128, 24) int32 index tile."""
    pv = pd_sb[:].bitcast(I32)
    nc.vector.scalar_tensor_tensor(out=pv, in0=pv, scalar=maskhi[:, 0:1],
                                   in1=iota[:],
                                   op0=mybir.AluOpType.bitwise_and,
                                   op1=mybir.AluOpType.bitwise_or)
    cand = sb.tile([P128, NCHUNK * 8], F32, tag="cand")
    for c in range(NCHUNK):
        nc.vector.max(out=cand[:, 8 * c:8 * c + 8], in_=pd_sb[:, CW * c:CW * (c + 1)])
    p24 = sb.tile([P128, 24], F32, tag="p24")
    for r in range(3):
        nc.vector.max(out=p24[:, 8 * r:8 * r + 8], in_=cand[:])
        if r < 2:
            nc.vector.match_replace(out=cand[:], in_to_replace=p24[:, 8 * r:8 * r + 8],
                                    in_values=cand[:], imm_value=-1e30)
    idx = sb.tile([P128, 24], I32, tag="idx")
    nc.vector.tensor_tensor(out=idx[:], in0=p24[:].bitcast(I32), in1=mask24[:],
                            op=mybir.AluOpType.bitwise_and)
    return idx


def _edgeconv_phase(nc, tc, pools, *, lhs, rhs, cdim, ptab_dram, bq_rhs,
                    bq_cdim, lhsq, out_cb, identity, iota, mask24, maskhi):
    """One EdgeConv: for each of NT query tiles compute pd (f32r matmul),
    packed top-20, one batched gather of P rows, max over k, add Q, LReLU.
    out_cb(t, xt) is called with the (128, 65) query-major result tile."""
    sb, ps, pd_ps, stg = pools
    for t in range(NT):
        q0 = t * P128
        # pd in 4 psum quarters of 1024 cols -> sbuf
        pd_sb = sb.tile([P128, N], F32, tag="pd_sb")
        for quar in range(4):
            pq = pd_ps.tile([P128, 1024], F32, tag="pdq")
            for j in range(2):
                c0 = quar * 1024 + j * 512
                nc.tensor.matmul(pq[:, j * 512:(j + 1) * 512],
                                 lhs[0:cdim, q0:q0 + P128],
                                 rhs[0:cdim, c0:c0 + 512],
                                 start=True, stop=True)
            nc.scalar.copy(pd_sb[:, quar * 1024:(quar + 1) * 1024], pq[:])
        idx = _knn_topk(nc, sb, pd_sb, iota, mask24, maskhi)
        # gather K neighbor rows of P^T (N, 64) per query; the HW indirect
        # DMA consumes ONE index per partition, so one DMA per k
        gath = sb.tile([P128, K * 64], F32, tag="gath")
        for k in range(K):
            nc.gpsimd.indirect_dma_start(
                out=gath[:, k * 64:(k + 1) * 64],
                out_offset=None,
                in_=ptab_dram[:],
                in_offset=bass.IndirectOffsetOnAxis(ap=idx[:, k:k + 1], axis=0),
            )
        # max over k: view gath (p, k, o) as (p, o, k), reduce innermost k
        m = sb.tile([P128, 64], F32, tag="gmx", name="gmx")
        nc.vector.tensor_reduce(
            out=m[:],
            in_=gath[:].rearrange("p (k o) -> p o k", o=64),
            axis=mybir.AxisListType.X, op=mybir.AluOpType.max)
        # Q part into psum, then accumulate the gather-max via identity matmul
        qp = ps.tile([P128, 64], F32, tag="ps")
        nc.tensor.matmul(qp[:], lhsq[0:bq_cdim, q0:q0 + P128], bq_rhs[0:bq_cdim, :],
                         start=True, stop=False)
        nc.tensor.matmul(qp[:], identity[:], m[:], start=False, stop=True)
        xt = sb.tile([P128, 65], F32, tag="xoq", name="xoq")
        nc.scalar.copy(xt[:, 0:64], qp[:])
        _lrelu(nc, xt[:, 0:64], xt[:, 0:64])
        out_cb(t, xt)


def _ptable(nc, pools, a_rhs, cdim, src, identity, ptab_dram, ptab_stage):
    """P = A^T-weights @ src -> transpose -> DRAM table (N, 64).
    a_rhs: (cdim, 64) stationary; src: (cdim, N)."""
    sb, ps, pd_ps, stg = pools
    for j in range(N // 512):
        pp = ps.tile([64, 512], F32, tag="ps")
        nc.tensor.matmul(pp[:], a_rhs[0:cdim, :], src[0:cdim, j * 512:(j + 1) * 512],
                         start=True, stop=True)
        p512 = stg.tile([64, 512], F32, tag="p512", name="p512")
        nc.scalar.copy(p512[:], pp[:])
        for i in range(4):
            j1 = 4 * j + i
            tp = ps.tile([P128, P128], F32, tag="ps")
            nc.tensor.transpose(tp[:, 0:64], p512[:, i * P128:(i + 1) * P128],
                                identity[0:64, 0:64])
            nc.scalar.copy(ptab_stage[:, j1 * 64:(j1 + 1) * 64], tp[:, 0:64])
    nc.sync.dma_start(
        ptab_dram[:].rearrange("(j1 j0) o -> j0 j1 o", j0=P128),
        ptab_stage[:].rearrange("p (j1 o) -> p j1 o", o=64))


def build():
    nc = bacc.Bacc(None, target_bir_lowering=False)
    dt = F32
    # ---- per-core inputs ----
    lhsA_d = nc.dram_tensor("lhsA_d", [7, HALF], dt, kind="ExternalInput")
    rhsA_d = nc.dram_tensor("rhsA_d", [7, N], dt, kind="ExternalInput")
    ones_d = nc.dram_tensor("ones_d", [2, N], dt, kind="ExternalInput")
    a1t = nc.dram_tensor("a1t", [2, 64], dt, kind="ExternalInput")
    b1t = nc.dram_tensor("b1t", [7, 64], dt, kind="ExternalInput")
    a2t = nc.dram_tensor("a2t", [64, 64], dt, kind="ExternalInput")
    b2t = nc.dram_tensor("b2t", [66, 64], dt, kind="ExternalInput")
    w5t = nc.dram_tensor("w5t", [128, 128], dt, kind="ExternalInput")
    t5 = nc.dram_tensor("t5", [128, 1], dt, kind="ExternalInput")
    w6at = nc.dram_tensor("w6at", [128, 256], dt, kind="ExternalInput")
    w6bt = nc.dram_tensor("w6bt", [128, 256], dt, kind="ExternalInput")
    t6 = nc.dram_tensor("t6", [128, 2], dt, kind="ExternalInput")
    w9t = nc.dram_tensor("w9t", [128, 2], dt, kind="ExternalInput")
    out = nc.dram_tensor("out", [1, HALF], dt, kind="ExternalOutput")

    with tile.TileContext(nc) as tc:
        with tc.tile_pool(name="sb", bufs=2) as sb, \
             tc.tile_pool(name="stg", bufs=1) as stg, \
             tc.tile_pool(name="sbp", bufs=1) as sbp, \
             tc.tile_pool(name="ps", bufs=2, space="PSUM") as ps, \
             tc.tile_pool(name="pdps", bufs=3, space="PSUM") as pd_ps, \
             tc.tile_pool(name="dram", bufs=1, space="DRAM") as dram:
            pools = (sb, ps, pd_ps, stg)
            ident = sbp.tile([P128, P128], F32, tag="ident")
            make_identity(nc, ident[:])
            iota = sbp.tile([P128, N], I32, tag="iota")
            nc.gpsimd.iota(iota[:], pattern=[[1, N]], base=0,
                           channel_multiplier=0)
            mask24 = sbp.tile([P128, 24], I32, tag="mask24")
            nc.vector.memset(mask24[:], 4095)
            maskhi = sbp.tile([P128, 1], I32, tag="maskhi")
            nc.gpsimd.iota(maskhi[:], pattern=[[1, 1]], base=-4096,
                           channel_multiplier=0)

            # ---------- phase A prep ----------
            # pd(q,m) = sum_c lhs_r[c,q]*rhs_r[c,m], cdim=7, rows built on
            # the HOST: lhsA_d = [2x_q; -x_q^2; 1;1;1],
            # rhsA_d = [x_m; 1; 1; -x_m^2; -1]
            # (the -1 keeps every pd <= -1: normal floats, packable)
            lhsA = sbp.tile([66, HALF], F32, tag="lhsAB")
            rhsA = sbp.tile([66, N], F32, tag="rhsAB")
            nc.sync.dma_start(lhsA[0:7, :], lhsA_d[:])
            nc.sync.dma_start(rhsA[0:7, :], rhsA_d[:])


            # ---------- load small weights (after the x-critical chain) ----
            def load(pool, dr, shape, tag):
                t = pool.tile(list(shape), F32, tag=tag)
                nc.sync.dma_start(t[:], dr[:])
                return t

            a1s = load(sbp, a1t, (2, 64), "a1")
            b1s = load(sbp, b1t, (7, 64), "b1")
            a2s = load(sbp, a2t, (64, 64), "a2")
            b2s = load(sbp, b2t, (66, 64), "b2")
            w5s = load(sbp, w5t, (128, 128), "w5")
            t5s = load(sbp, t5, (128, 1), "t5")
            w6as = load(sbp, w6at, (128, 256), "w6a")
            w6bs = load(sbp, w6bt, (128, 256), "w6b")
            t6s = load(sbp, t6, (128, 2), "t6")
            w9s = load(sbp, w9t, (128, 2), "w9")

            # ---------- P1 table ----------
            p1d = dram.tile([N, 64], F32, tag="p1d")
            pstage = stg.tile([P128, (N // P128) * 64], F32, tag="pstage")
            _ptable(nc, pools, a1s, 2, rhsA, ident, p1d, pstage)

            # ---------- EdgeConv 1 ----------
            # x1 q-major -> (sq-norm col, with extra -1) -> transpose into
            # x1aug (66, HALF)
            x1aug = sbp.tile([66, HALF], F32, tag="x1aug")

            def cb1(t, xt):
                sq = sb.tile([P128, 64], F32, tag="sq64", name="sq64")
                nc.vector.tensor_mul(sq[:], xt[:, 0:64], xt[:, 0:64])
                nc.vector.tensor_reduce(out=xt[:, 64:65], in_=sq[:],
                                        axis=mybir.AxisListType.X,
                                        op=mybir.AluOpType.add, negate=True)
                nc.vector.tensor_scalar_add(xt[:, 64:65], xt[:, 64:65], -1.0)
                tp = ps.tile([P128, P128], F32, tag="ps", name="tp1")
                nc.tensor.transpose(tp[0:65, :], xt[:, 0:65], ident[:])
                nc.scalar.copy(x1aug[0:65, t * P128:(t + 1) * P128], tp[0:65, :])

            _edgeconv_phase(nc, tc, pools, lhs=lhsA, rhs=rhsA, cdim=7,
                            ptab_dram=p1d, bq_rhs=b1s, bq_cdim=7, lhsq=lhsA,
                            out_cb=cb1, identity=ident, iota=iota, mask24=mask24, maskhi=maskhi)
            nc.sync.dma_start(x1aug[65:66, :], ones_d[0:1, 0:HALF])

            # ---------- exchange halves (AllGather over core pairs) ----------
            # bf16 exchange: halves collective + staging time; x1 reaches the
            # partner at bf16 precision (KNN-2 + P2 tolerance is ample)
            BF16 = mybir.dt.bfloat16
            x1bf = stg.tile([66, HALF], BF16, tag="x1bf")
            nc.scalar.copy(x1bf[0:66, :], x1aug[0:66, :])
            ccin = dram.tile([66, HALF], BF16, tag="ccin")
            ccout = dram.tile([132, HALF], BF16, tag="ccout")
            nc.sync.dma_start(ccin[:], x1bf[0:66, :])
            nc.gpsimd.collective_compute(
                "AllGather", mybir.AluOpType.bypass,
                replica_groups=[[0, 1], [2, 3], [4, 5], [6, 7]],
                ins=[ccin.opt()], outs=[ccout.opt()])

            # ---------- phase B prep ----------
            # collective-independent work first so engines stay busy during
            # the exchange: lhsB = [2*x1; norm; ones] via Act scale-2 copy
            lhsB = sbp.tile([66, HALF], F32, tag="lhsAB")
            nc.scalar.activation(lhsB[0:64, :], x1aug[0:64, :],
                                 mybir.ActivationFunctionType.Copy, scale=2.0)
            nc.scalar.copy(lhsB[64:66, :], x1aug[64:66, :])
            xcown = sbp.tile([P128, HALF], F32, tag="xcown")
            nc.scalar.copy(xcown[0:64, :], x1aug[0:64, :])

            # rhsB (66, N): [x1_full; ones; -xx2_full - 1]  (reuses rhsA slot)
            # pull the bf16 exchange into SBUF, convert to fp32 (Act), and
            # route the norm rows through partition-aligned staging + DMA
            xbf = stg.tile([66, N], BF16, tag="xbf")
            nc.sync.dma_start(xbf[0:65, 0:HALF], ccout[0:65, :])
            nc.scalar.dma_start(xbf[0:65, HALF:N], ccout[66:131, :])
            rhsB = sbp.tile([66, N], F32, tag="rhsAB")
            nc.scalar.copy(rhsB[0:64, :], xbf[0:64, :])
            nrm_f = stg.tile([65, N], F32, tag="nrmf")
            nc.scalar.copy(nrm_f[64:65, :], xbf[64:65, :])
            nc.sync.dma_start(rhsB[65:66, :], nrm_f[64:65, :])
            nc.sync.dma_start(rhsB[64:65, :], ones_d[0:1, :])

            # ---------- P2 table ----------
            p2d = dram.tile([N, 64], F32, tag="p2d")
            _ptable(nc, pools, a2s, 64, rhsB, ident, p2d, pstage)

            # conv5 + global max pool is fused per 512-chunk into cb2:
            # chunk j runs as soon as query tiles 4j..4j+3 have landed
            gpart = stg.tile([P128, 4], F32, tag="gpart")

            def conv5_chunk(j):
                pp = ps.tile([P128, 512], F32, tag="ps")
                nc.tensor.matmul(pp[:], w5s[:], xcown[:, j * 512:(j + 1) * 512],
                                 start=True, stop=True)
                h5c = sb.tile([P128, 512], F32, tag="h5c", name="h5c")
                nc.scalar.activation(h5c[:], pp[:],
                                     mybir.ActivationFunctionType.Identity,
                                     bias=t5s[:, 0:1])
                _lrelu(nc, h5c[:], h5c[:])
                nc.vector.tensor_reduce(out=gpart[:, j:j + 1], in_=h5c[:],
                                        axis=mybir.AxisListType.X,
                                        op=mybir.AluOpType.max)

            def cb2(t, xt):
                tp = ps.tile([P128, P128], F32, tag="ps", name="tp2")
                nc.tensor.transpose(tp[0:64, :], xt[:, 0:64], ident[:])
                x2s = sb.tile([64, P128], F32, tag="x2s", name="x2s")
                nc.scalar.copy(x2s[:], tp[0:64, :])
                nc.sync.dma_start(xcown[64:128, t * P128:(t + 1) * P128], x2s[:])
                if t % 4 == 3:
                    conv5_chunk(t // 4)

            _edgeconv_phase(nc, tc, pools, lhs=lhsB, rhs=rhsB, cdim=66,
                            ptab_dram=p2d, bq_rhs=b2s, bq_cdim=66, lhsq=x1aug,
                            out_cb=cb2, identity=ident, iota=iota, mask24=mask24, maskhi=maskhi)

            # ---------- tail ----------
            gown = sb.tile([P128, 1], F32, tag="gown")
            nc.vector.tensor_reduce(out=gown[:], in_=gpart[:, 0:4],
                                    axis=mybir.AxisListType.X, op=mybir.AluOpType.max)
            gin = dram.tile([P128, 1], F32, tag="gin")
            gout = dram.tile([256, 1], F32, tag="gout")
            nc.sync.dma_start(gin[:], gown[:])
            nc.gpsimd.collective_compute(
                "AllGather", mybir.AluOpType.bypass,
                replica_groups=[[0, 1], [2, 3], [4, 5], [6, 7]],
                ins=[gin.opt()], outs=[gout.opt()])
            # W6b matmuls do not need gmax: issue them on the PE while the
            # collective runs, into wide psum tiles from the pd pool
            p6 = []
            for c in range(2):
                for jh in range(2):
                    pq6 = pd_ps.tile([P128, 1024], F32, tag="pdq",
                                     name=f"p6_{c}{jh}")
                    for j2 in range(2):
                        j = jh * 2 + j2
                        nc.tensor.matmul(pq6[:, j2 * 512:(j2 + 1) * 512],
                                         w6bs[:, c * 128:(c + 1) * 128],
                                         xcown[:, j * 512:(j + 1) * 512],
                                         start=True, stop=True)
                    p6.append(pq6)
            gboth = sb.tile([P128, 2], F32, tag="gboth")
            nc.sync.dma_start(gboth[:, 0:1], gout[0:P128, :])
            nc.sync.dma_start(gboth[:, 1:2], gout[P128:256, :])
            gmax = sb.tile([P128, 1], F32, tag="gmax")
            nc.vector.tensor_reduce(out=gmax[:], in_=gboth[:],
                                    axis=mybir.AxisListType.X, op=mybir.AluOpType.max)
            bias6 = sb.tile([P128, 2], F32, tag="bias6")
            for c in range(2):
                vp = ps.tile([P128, 64], F32, tag="ps")
                nc.tensor.matmul(vp[:, 0:1], w6as[:, c * 128:(c + 1) * 128], gmax[:],
                                 start=True, stop=True)
                nc.vector.tensor_add(out=bias6[:, c:c + 1], in0=vp[:, 0:1],
                                     in1=t6s[:, c:c + 1])
            h6t = []
            for c in range(2):
                for jh in range(2):
                    sl6 = sb.tile([P128, 1024], F32, tag=f"h6c{c}", name=f"h6c{c}")
                    nc.scalar.activation(sl6[:], p6[c * 2 + jh][:],
                                         mybir.ActivationFunctionType.Identity,
                                         bias=bias6[:, c:c + 1])
                    _lrelu(nc, sl6[:], sl6[:])
                    h6t.append(sl6)
            osb = sb.tile([1, HALF], F32, tag="osb")
            for jh in range(2):
                for j2 in range(2):
                    pp9 = ps.tile([1, 512], F32, tag="ps")
                    for c in range(2):
                        nc.tensor.matmul(pp9[:], w9s[:, c:c + 1],
                                         h6t[c * 2 + jh][:, j2 * 512:(j2 + 1) * 512],
                                         start=(c == 0), stop=(c == 1))
                    nc.scalar.copy(osb[:, (jh * 2 + j2) * 512:(jh * 2 + j2 + 1) * 512],
                                   pp9[:])
            nc.sync.dma_start(out[:], osb[:])

    nc.finalize()
    return nc


def _fold_weights(i):
    f = np.float32
    o = {}
    s1 = (i["g1"] / np.sqrt(i["v1"] + EPS)).astype(f)
    A1 = (i["W1"][:, 0:2] * s1[:, None]).astype(f)
    B1 = ((i["W1"][:, 2:4] - i["W1"][:, 0:2]) * s1[:, None]).astype(f)
    t1 = (i["b1"] - i["m1"] * s1).astype(f)
    o["a1t"] = np.ascontiguousarray(A1.T)
    b1t = np.zeros((7, 64), f)
    b1t[0:2] = 0.5 * B1.T
    b1t[4] = t1
    o["b1t"] = b1t
    s2 = (i["g2"] / np.sqrt(i["v2"] + EPS)).astype(f)
    A2 = (i["W2"][:, 0:64] * s2[:, None]).astype(f)
    B2 = ((i["W2"][:, 64:128] - i["W2"][:, 0:64]) * s2[:, None]).astype(f)
    t2 = (i["b2"] - i["m2"] * s2).astype(f)
    o["a2t"] = np.ascontiguousarray(A2.T)
    b2t = np.zeros((66, 64), f)
    b2t[0:64] = B2.T
    b2t[65] = t2
    o["b2t"] = b2t
    s5 = (i["g5"] / np.sqrt(i["v5"] + EPS)).astype(f)
    W5s = (i["W5"] * s5[:, None]).astype(f)
    o["w5t"] = np.ascontiguousarray(W5s.T)
    o["t5"] = (i["b5"] - i["m5"] * s5).astype(f).reshape(128, 1)
    s6 = (i["g6"] / np.sqrt(i["v6"] + EPS)).astype(f)
    W6s = (i["W6"] * s6[:, None]).astype(f)
    o["w6at"] = np.ascontiguousarray(W6s[:, 0:128].T)
    o["w6bt"] = np.ascontiguousarray(W6s[:, 128:256].T)
    t6 = (i["b6"] - i["m6"] * s6).astype(f)
    o["t6"] = np.ascontiguousarray(t6.reshape(2, 128).T)
    o["w9t"] = np.ascontiguousarray(i["W9"].reshape(2, 128).T)
    ones_d = np.ones((2, N), f)
    ones_d[1] = -1.0
    o["ones_d"] = ones_d
    return o


def _core_inputs(w, x, c):
    b, h = c // 2, c % 2
    m = dict(w)
    xb = x[b]
    xqh = xb[:, h * HALF:(h + 1) * HALF]
    lhsA_d = np.empty((7, HALF), np.float32)
    lhsA_d[0:2] = 2.0 * xqh
    lhsA_d[2:4] = -(xqh * xqh)
    lhsA_d[4:7] = 1.0
    rhsA_d = np.empty((7, N), np.float32)
    rhsA_d[0:2] = xb
    rhsA_d[2:4] = 1.0
    rhsA_d[4:6] = -(xb * xb)
    rhsA_d[6] = -1.0
    m["lhsA_d"] = lhsA_d
    m["rhsA_d"] = rhsA_d
    return m


def kernel(**inputs):
    inputs = {k: np.asarray(v, np.float32) for k, v in inputs.items()}
    if "nc" not in _BUILT:
        _BUILT["nc"] = build()
    nc = _BUILT["nc"]
    w = _fold_weights(inputs)
    x = inputs["x"]
    in_maps = [_core_inputs(w, x, c) for c in range(NCORES)]
    res = run_bass_kernel_spmd(nc, in_maps, core_ids=list(range(NCORES)))
    out = np.zeros((B, N), np.float32)
    for c in range(NCORES):
        b, h = c // 2, c % 2
        out[b, h * HALF:(h + 1) * HALF] = res.results[c]["out"][0]
    return out
